# revision 23
# baseline (speedup 1.0000x reference)
"""Trainium2 Bass kernel for nn_DripBlock: per-sample modulated 3x3 conv.

Math (per sample b):
  s = w @ (linear_w / sqrt(WDIM)).T + linear_b                  [b, in_c]
  base_w = conv_w / sqrt(in_c*3*3)
  wmod = base_w * s[:,None,:,None,None]
  sigma_inv = rsqrt(sum(wmod^2, (in,ky,kx)) + 1e-8)             [b, out]
  y = conv2d(x, wmod*sigma_inv, SAME) + scale_noise*noise + bias
  out = leaky_relu(y, 0.2)

Kernel strategy (data-parallel over batch, 2 samples/core on 8 cores):
  - Fold s into x (xs = x*s per channel); conv against raw conv_w; fold
    C1*sigma_inv, bias, noise into the post ops.
  - 1D Winograd F(2,3) along W; weights stored HALVED (cast scale 0.5)
    so U1=(w0+w1+w2)/2 and U2=(w0-w1+w2)/2 are plain adds of the halved
    taps; the 2x compensation for the U0/U3 planes is folded into the
    drain's scalar_tensor_tensor scalars.
  - Drains use at most one PSUM operand per instruction (PSUM has one
    DVE read port): ScalarE copies M1 to SBUF; DVE computes
    t=2*M0+m1, e=t+M2, t2=m1-M2, o=-2*M3+t2 and z2=ssc*e/o+pre;
    ScalarE computes pre=sn*noise+bias and the leaky-relu (Lrelu
    activation, alpha=0.2) writing interleaved even/odd columns.
  - sigma via tap-reduced squared weights: sqw=wchunk^2 (ScalarE
    Square), wsq[ic,oc]=sum_tap sqw (DVE strided reduce), then 4 tiny
    f32 matmuls against s^2 per oc chunk.
  - Schedule: occ-OUTER over band pairs so weight emission for occ k+1
    overlaps the two groups of occ k; conv_w rides the scalar HW queue,
    x / xbar-transposes / outputs ride the sync queue; small constant
    vectors are loaded as [4,128] rows and PE-transposed (tiny-gather
    DMAs are slow).  The occ0 prologue interleaves per-icc
    cast/transpose/x-load so the first conv matmul starts ~16us in and
    the per-icc weight chain stays just ahead of the PE.
"""
import numpy as np
from math import sqrt
from contextlib import ExitStack

import concourse.bass as bass
import concourse.bacc as bacc
import concourse.mybir as mybir
import concourse.tile as tile
from concourse.masks import make_identity

B, CIN, COUT, H, W, WDIM, KK = 16, 512, 512, 64, 64, 512, 3
NCORES = 8
BLOC = B // NCORES          # 2 samples per core
P = 128
NIC = CIN // P              # 4 ic chunks
NOC = COUT // P             # 4 oc chunks
NDC = WDIM // P             # 4 wdim chunks
NBAND = 4                   # 16-row bands per sample
RB = H // NBAND             # 16 rows per band
WT = W // 2                 # 32 column tiles (2 output cols each)
XR = RB + 2                 # 18 staged rows per band
EPS = 1e-8
C0 = 1.0 / sqrt(WDIM)
C1 = 1.0 / sqrt(CIN * KK * KK)
SLOPE = 0.2

F32 = mybir.dt.float32
BF16 = mybir.dt.bfloat16
MUL = mybir.AluOpType.mult
ADD = mybir.AluOpType.add
SUB = mybir.AluOpType.subtract
MAX = mybir.AluOpType.max
COPYF = mybir.ActivationFunctionType.Copy
SQRTF = mybir.ActivationFunctionType.Sqrt
IDENTF = mybir.ActivationFunctionType.Identity
LRELUF = mybir.ActivationFunctionType.Lrelu
# The ScalarE Lrelu activation ignores alpha on HW (acts as plain Relu),
# so leaky-relu runs on DVE via max(x, 0.2x).
USE_ACT_LRELU = False


def build_nc():
    nc = bacc.Bacc()

    x_d = nc.declare_dram_parameter("x", [BLOC, CIN, H, W], F32, isOutput=False)
    w_d = nc.declare_dram_parameter("w", [BLOC, WDIM], F32, isOutput=False)
    noise_d = nc.declare_dram_parameter("noise", [BLOC, 1, H, W], F32, isOutput=False)
    lw_d = nc.declare_dram_parameter("linear_w", [CIN, WDIM], F32, isOutput=False)
    lb_d = nc.declare_dram_parameter("linear_b", [CIN], F32, isOutput=False)
    cw_d = nc.declare_dram_parameter("conv_w", [COUT, CIN, KK, KK], F32, isOutput=False)
    sn_d = nc.declare_dram_parameter("scale_noise", [COUT], F32, isOutput=False)
    bias_d = nc.declare_dram_parameter("bias", [COUT], F32, isOutput=False)
    out_d = nc.declare_dram_parameter("out", [BLOC, COUT, H, W], F32, isOutput=True)

    with ExitStack() as ctx:
        tc = ctx.enter_context(tile.TileContext(nc))
        consts = ctx.enter_context(tc.tile_pool(name="consts", bufs=1))
        lw_pool = ctx.enter_context(tc.tile_pool(name="lw", bufs=2))
        lwt_pool = ctx.enter_context(tc.tile_pool(name="lwt", bufs=4))
        co_pool = ctx.enter_context(tc.tile_pool(name="co", bufs=4))
        cobf_pool = ctx.enter_context(tc.tile_pool(name="cobf", bufs=2))
        wt_pool = ctx.enter_context(tc.tile_pool(name="wt", bufs=1))
        u_pool = ctx.enter_context(tc.tile_pool(name="u", bufs=1))
        ua_pool = ctx.enter_context(tc.tile_pool(name="ua", bufs=1))
        sqw_pool = ctx.enter_context(tc.tile_pool(name="sqw", bufs=2))
        wsq_pool = ctx.enter_context(tc.tile_pool(name="wsq", bufs=1))
        small = ctx.enter_context(tc.tile_pool(name="small", bufs=1))
        xt_pool = ctx.enter_context(tc.tile_pool(name="xt", bufs=2))
        v_pool = ctx.enter_context(tc.tile_pool(name="v", bufs=1))
        nb_pool = ctx.enter_context(tc.tile_pool(name="nb", bufs=1))
        dr_pool = ctx.enter_context(tc.tile_pool(name="dr", bufs=2))
        pz_pool = ctx.enter_context(tc.tile_pool(name="pz", bufs=2))
        out_pool = ctx.enter_context(tc.tile_pool(name="out", bufs=2))

        psum = ctx.enter_context(tc.tile_pool(name="mmps", bufs=8, space="PSUM"))

        # ---- conv_w occ0 loads first on the scalar HW queue ----
        co_tiles = {}

        def load_co(occ):
            for icc in range(NIC):
                co = co_pool.tile([P, P * KK * KK], F32, tag="co",
                                  name=f"co{icc}_{occ}")
                nc.scalar.dma_start(
                    out=co,
                    in_=cw_d[occ * P:(occ + 1) * P, icc * P:(icc + 1) * P, :, :]
                    .rearrange("o i a b -> o (i a b)"))
                co_tiles[(icc, occ)] = co

        load_co(0)

        # ---- constants: [4,128] row loads on the sync queue + PE transpose
        # (element-gather DMAs are packet-bound and slow) ----
        ident = consts.tile([P, P], F32)
        make_identity(nc, ident)
        eps_col = consts.tile([P, 1], F32)
        nc.vector.memset(eps_col, EPS)

        # PE warmup: ~4us of dummy matmuls flips the HAM clock gate to 8/8
        # before the real conv stream starts (transposes don't count as
        # PE-busy for HAM).
        wm = consts.tile([P, P], BF16)
        nc.vector.memset(wm, 0.0)
        wm_ps = psum.tile([P, P], F32, tag="mm", name="warm")
        for _ in range(40):
            nc.tensor.matmul(wm_ps, wm, wm, start=True, stop=True)

        # lw on the sync queue so s is ready early
        lw_sbs = []
        for icc in range(NIC):
            lw_sb = lw_pool.tile([P, WDIM], F32, tag="lw", bufs=3,
                                 name=f"lw{icc}")
            nc.sync.dma_start(out=lw_sb, in_=lw_d[icc * P:(icc + 1) * P, :])
            lw_sbs.append(lw_sb)

        rows_pool = ctx.enter_context(tc.tile_pool(name="rows", bufs=2))

        def rowload_cols(src_1d, n, out_ap=None, tag="cols"):
            # src [n*P] -> rows [n, P] -> PE transpose -> [P, n]
            rows = rows_pool.tile([n, P], F32, name="rows", tag="rows")
            nc.sync.dma_start(out=rows, in_=src_1d.rearrange("(c p) -> c p", p=P))
            tp = psum.tile([P, n], F32, tag="mm", name="tpc")
            nc.tensor.transpose(tp, rows, ident[0:n, 0:n])
            if out_ap is None:
                out_ap = consts.tile([P, n], F32, name=tag, tag=tag)
            nc.vector.tensor_copy(out=out_ap, in_=tp)
            return out_ap

        lb_cols = rowload_cols(lb_d[:], NIC, tag="lbc")
        bias_cols = rowload_cols(bias_d[:], NOC, tag="bic")
        sn_cols = rowload_cols(sn_d[:], NOC, tag="snc")
        wcols = consts.tile([P, NDC, BLOC], F32)
        for b in range(BLOC):
            rowload_cols(w_d[b], NDC, out_ap=wcols[:, :, b])

        # ---- phase A: s = w @ (linear_w*C0).T + linear_b, as sT[ic, b] ----
        sT = []
        s2T = []
        for icc in range(NIC):
            lwt = []
            for dc in range(NDC):
                tp = psum.tile([P, P], F32, tag="mm", name="tp")
                nc.tensor.transpose(tp, lw_sbs[icc][:, dc * P:(dc + 1) * P], ident)
                t_ = lwt_pool.tile([P, P], F32, tag="lwt")
                nc.vector.tensor_copy(out=t_, in_=tp)
                lwt.append(t_)
            sp = psum.tile([P, BLOC], F32, tag="mm", name=f"sp{icc}")
            for dc in range(NDC):
                nc.tensor.matmul(sp, lwt[dc], wcols[:, dc, :],
                                 start=(dc == 0), stop=(dc == NDC - 1))
            st = small.tile([P, BLOC], F32, tag=f"sT{icc}")
            nc.vector.tensor_scalar(out=st, in0=sp, scalar1=C0,
                                    scalar2=lb_cols[:, icc:icc + 1],
                                    op0=MUL, op1=ADD)
            s2 = small.tile([P, BLOC], F32, tag=f"s2T{icc}")
            nc.vector.tensor_mul(s2, st, st)
            sT.append(st)
            s2T.append(s2)

        # ---- staging: x band -> scaled f32 xt -> V planes (bf16) ----
        vts = {}     # gb -> [vt per icc]
        nbs = {}     # gb -> [128, RB*W] f32 noise broadcast

        def stage_icc(gb, icc):
            b, band = divmod(gb, NBAND)
            slot = gb % 3
            r0 = band * RB
            xt = xt_pool.tile([P, XR, W], F32, tag="xt", name=f"xt{gb}_{icc}")
            lo = max(r0 - 1, 0)
            hi = min(r0 - 1 + XR, H)
            j0 = lo - (r0 - 1)
            if j0 > 0:
                nc.vector.memset(xt[:, 0:j0, :], 0.0)
            if (r0 - 1 + XR) > H:
                nc.vector.memset(xt[:, XR - 1:XR, :], 0.0)
            nc.sync.dma_start(
                out=xt[:, j0:j0 + (hi - lo), :],
                in_=x_d[b, icc * P:(icc + 1) * P, lo:hi, :])
            nc.scalar.activation(out=xt, in_=xt, func=COPYF,
                                 scale=sT[icc][:, b:b + 1])
            vt = v_pool.tile([P, 4, XR, WT], BF16, tag=f"v{icc}s{slot}",
                             name=f"v{gb}_{icc}")
            xte = xt.rearrange("p r (w two) -> p r w two", two=2)
            ev = xte[:, :, :, 0]   # x cols 0,2,..,62
            od = xte[:, :, :, 1]   # x cols 1,3,..,63
            # V0[k] = d(2k-1) - d(2k+1);  k=0: -x[1]          (DVE)
            nc.vector.tensor_tensor(out=vt[:, 0, :, 1:WT], in0=od[:, :, 0:WT - 1],
                                    in1=od[:, :, 1:WT], op=SUB)
            nc.vector.tensor_scalar_mul(out=vt[:, 0, :, 0:1], in0=od[:, :, 0:1],
                                        scalar1=-1.0)
            # V1/V2/V3 main bodies on GpSimd
            nc.gpsimd.tensor_tensor(out=vt[:, 1], in0=ev, in1=od, op=ADD)
            nc.gpsimd.tensor_tensor(out=vt[:, 2], in0=od, in1=ev, op=SUB)
            nc.gpsimd.tensor_tensor(out=vt[:, 3, :, 0:WT - 1], in0=ev[:, :, 0:WT - 1],
                                    in1=ev[:, :, 1:WT], op=SUB)
            nc.vector.tensor_copy(out=vt[:, 3, :, WT - 1:WT], in_=ev[:, :, WT - 1:WT])
            return vt

        def stage_band(gb, skip_icc=0):
            b, band = divmod(gb, NBAND)
            vt_l = vts.get(gb, [])
            for icc in range(skip_icc, NIC):
                vt_l.append(stage_icc(gb, icc))
            vts[gb] = vt_l
            r0 = band * RB
            nb = nb_pool.tile([P, RB * W], F32, tag=f"nb{gb % 3}", name=f"nb{gb}")
            nsrc = noise_d[b].rearrange("o h w -> o (h w)")[0:1, r0 * W:(r0 + RB) * W]
            nsrc_bc = bass.AP(tensor=nsrc.tensor, offset=nsrc.offset,
                              ap=[[0, P]] + list(nsrc.ap)[1:])
            nc.scalar.dma_start(out=nb, in_=nsrc_bc)
            nbs[gb] = nb

        # ---- weights: wchunk[ic, tap, oc] (halved), u1/u2, sqw/wsq ----
        wchunks = {}
        u1s = {}
        u2s = {}
        wsqs = {}
        sig_scale = {}   # occ -> [P, BLOC] f32: C1*sigma_inv

        def emit_weight_chunk(icc, occ):
            co = co_tiles.pop((icc, occ))
            # cast to bf16 in tap-major order with the Winograd 1/2 folded in
            co_bf = cobf_pool.tile([P, KK * KK * P], BF16, tag="cobf")
            nc.scalar.activation(
                out=co_bf.rearrange("o (n i) -> o n i", i=P),
                in_=co.rearrange("o (i n) -> o n i", n=KK * KK),
                func=COPYF, scale=0.5)
            wchunk = wt_pool.tile([P, KK * KK, P], BF16,
                                  tag=f"wt{icc}_{occ}", name=f"wt{icc}_{occ}")
            nc.sync.dma_start_transpose(out=wchunk, in_=co_bf)
            wchunks[(icc, occ)] = wchunk
            # U1/U2 = (g0 +- g1 + g2)/2 of the true taps (halved taps add)
            wc4 = wchunk.rearrange("i (ky kx) o -> i ky kx o", kx=KK)
            w0 = wc4[:, :, 0, :]
            w1 = wc4[:, :, 1, :]
            w2 = wc4[:, :, 2, :]
            ua = ua_pool.tile([P, KK, P], BF16, tag="ua")
            nc.gpsimd.tensor_tensor(out=ua, in0=w0, in1=w2, op=ADD)
            u1 = u_pool.tile([P, KK, P], BF16, tag=f"u1_{icc}_{occ}",
                             name=f"u1_{icc}_{occ}")
            nc.gpsimd.tensor_tensor(out=u1, in0=ua, in1=w1, op=ADD)
            u2 = u_pool.tile([P, KK, P], BF16, tag=f"u2_{icc}_{occ}",
                             name=f"u2_{icc}_{occ}")
            ueng = nc.gpsimd if icc < 2 else nc.vector
            ueng.tensor_tensor(out=u2, in0=ua, in1=w1, op=SUB)
            u1s[(icc, occ)] = u1
            u2s[(icc, occ)] = u2

        def emit_sigma_prep(icc, occ):
            # sqw = wchunk^2 (0.25*w^2) on ScalarE, written [ic, oc, tap] so
            # the DVE tap-reduce reads contiguously
            sqw = sqw_pool.tile([P, P, KK * KK], BF16, tag="sqw")
            nc.scalar.activation(out=sqw.rearrange("i o t -> i t o"),
                                 in_=wchunks[(icc, occ)],
                                 func=mybir.ActivationFunctionType.Square)
            wsq = wsq_pool.tile([P, P], F32, tag=f"wsq{icc}", name=f"wsq{icc}_{occ}")
            nc.vector.tensor_reduce(out=wsq, in_=sqw,
                                    axis=mybir.AxisListType.X, op=ADD)
            wsqs[(icc, occ)] = wsq

        def emit_weights(occ):
            for icc in range(NIC):
                emit_weight_chunk(icc, occ)
            if occ + 1 < NOC:
                load_co(occ + 1)
            for icc in range(NIC):
                emit_sigma_prep(icc, occ)

        def emit_sigma(occ):
            sg = psum.tile([P, BLOC], F32, tag="mm", name=f"sg{occ}")
            for icc in range(NIC):
                nc.tensor.matmul(sg, wsqs.pop((icc, occ)), s2T[icc],
                                 start=(icc == 0), stop=(icc == NIC - 1))
            # sg = 0.25 * sum w^2 s^2; sig = sqrt(C1^2 * 4 * sg + EPS)
            sig = small.tile([P, BLOC], F32, tag=f"sig{occ}")
            nc.scalar.activation(out=sig, in_=sg, func=SQRTF,
                                 bias=eps_col[:, 0:1], scale=4.0 * C1 * C1)
            sinv = small.tile([P, BLOC], F32, tag=f"sinv{occ}")
            nc.vector.reciprocal(out=sinv, in_=sig)
            ssc = small.tile([P, BLOC], F32, tag=f"ssc{occ}")
            nc.vector.tensor_scalar_mul(out=ssc, in0=sinv, scalar1=C1)
            sig_scale[occ] = ssc

        # ---- conv group + post ----
        out3 = out_d.rearrange("b c h w -> b c (h w)")

        def group(b, band, occ):
            gb = b * NBAND + band
            vt_l = vts[gb]
            nb_t = nbs[gb]
            M = [psum.tile([P, RB * WT], F32, tag="mm", name=f"M{xi}")
                 for xi in range(4)]
            for icc in range(NIC):
                vt = vt_l[icc]
                wc = wchunks[(icc, occ)]
                for ky in range(KK):
                    lhs = (wc[:, 3 * ky, :],
                           u1s[(icc, occ)][:, ky, :],
                           u2s[(icc, occ)][:, ky, :],
                           wc[:, 3 * ky + 2, :])
                    st = (icc == 0 and ky == 0)
                    sp = (icc == NIC - 1 and ky == KK - 1)
                    for xi in range(4):
                        nc.tensor.matmul(
                            M[xi], lhs[xi], vt[:, xi, ky:ky + RB, :],
                            start=st, stop=sp)
            # drains: one PSUM operand per op.  ScalarE: m1 = M1 -> SBUF.
            m1 = dr_pool.tile([P, RB * WT], F32, tag="m1", name="m1")
            nc.scalar.activation(out=m1, in_=M[1], func=COPYF)
            # DVE: t = 2*M0 + m1 ; e = t + M2 ; t2 = m1 - M2 ; o = -2*M3 + t2
            t = dr_pool.tile([P, RB * WT], F32, tag="t", name="t", bufs=1)
            nc.vector.scalar_tensor_tensor(out=t, in0=M[0], scalar=2.0,
                                           in1=m1, op0=MUL, op1=ADD)
            e = dr_pool.tile([P, RB * WT], F32, tag="e", name="e")
            nc.vector.tensor_tensor(out=e, in0=t, in1=M[2], op=ADD)
            t2 = dr_pool.tile([P, RB * WT], F32, tag="t2", name="t2", bufs=1)
            nc.vector.tensor_tensor(out=t2, in0=m1, in1=M[2], op=SUB)
            o = dr_pool.tile([P, RB * WT], F32, tag="o", name="o")
            nc.vector.scalar_tensor_tensor(out=o, in0=M[3], scalar=-2.0,
                                           in1=t2, op0=MUL, op1=ADD)
            # ScalarE: pre = sn*noise + bias ; DVE: z2 = ssc*e/o + pre
            ssc = sig_scale[occ][:, b:b + 1]
            pre = pz_pool.tile([P, RB, W], BF16, tag="pre", name="pre")
            nc.scalar.activation(out=pre,
                                 in_=nb_t.rearrange("p (r w) -> p r w", r=RB),
                                 func=IDENTF,
                                 scale=sn_cols[:, occ:occ + 1],
                                 bias=bias_cols[:, occ:occ + 1])
            prev = pre.rearrange("p r (w two) -> p r w two", two=2)
            ev3 = e.rearrange("p (r w) -> p r w", w=WT)
            ov3 = o.rearrange("p (r w) -> p r w", w=WT)
            z2e = pz_pool.tile([P, RB, WT], BF16, tag="z2e", name="z2e")
            nc.vector.scalar_tensor_tensor(out=z2e, in0=ev3, scalar=ssc,
                                           in1=prev[:, :, :, 0], op0=MUL, op1=ADD)
            z2o = pz_pool.tile([P, RB, WT], BF16, tag="z2o", name="z2o")
            nc.vector.scalar_tensor_tensor(out=z2o, in0=ov3, scalar=ssc,
                                           in1=prev[:, :, :, 1], op0=MUL, op1=ADD)
            # leaky relu, interleaving even/odd back to row-major
            zo = out_pool.tile([P, RB, WT, 2], F32, tag="zo", name="zo")
            if USE_ACT_LRELU:
                nc.scalar.activation(out=zo[:, :, :, 0], in_=z2e, func=LRELUF,
                                     alpha=SLOPE)
                nc.scalar.activation(out=zo[:, :, :, 1], in_=z2o, func=LRELUF,
                                     alpha=SLOPE)
            else:
                nc.vector.scalar_tensor_tensor(out=zo[:, :, :, 0], in0=z2e,
                                               scalar=SLOPE, in1=z2e,
                                               op0=MUL, op1=MAX)
                nc.vector.scalar_tensor_tensor(out=zo[:, :, :, 1], in0=z2o,
                                               scalar=SLOPE, in1=z2o,
                                               op0=MUL, op1=MAX)
            nc.sync.dma_start(
                out=out3[b, occ * P:(occ + 1) * P,
                         band * RB * W:(band + 1) * RB * W],
                in_=zo.rearrange("p r w two -> p (r w two)"))

        # ---- prologue: interleave occ0 weight chain with band0 staging so
        # the per-icc matmul chain starts as soon as chunk icc lands ----
        vts[0] = []
        for icc in range(NIC):
            vts[0].append(stage_icc(0, icc))
            emit_weight_chunk(icc, 0)
        r0nb = nb_pool.tile([P, RB * W], F32, tag="nb0", name="nb0")
        nsrc = noise_d[0].rearrange("o h w -> o (h w)")[0:1, 0:RB * W]
        nc.scalar.dma_start(out=r0nb, in_=bass.AP(
            tensor=nsrc.tensor, offset=nsrc.offset,
            ap=[[0, P]] + list(nsrc.ap)[1:]))
        nbs[0] = r0nb
        stage_band(1)
        load_co(1)
        for icc in range(NIC):
            emit_sigma_prep(icc, 0)

        # ---- main loop: occ-outer over band pairs ----
        for b in range(BLOC):
            for pp in range(NBAND // 2):
                bnd0 = 2 * pp
                g0 = b * NBAND + bnd0
                for occ in range(NOC):
                    if b == 0 and pp == 0:
                        emit_sigma(occ)
                    group(b, bnd0, occ)
                    if b == 0 and pp == 0 and occ + 1 < NOC:
                        emit_weights(occ + 1)
                    if occ == 1 and g0 + 2 < BLOC * NBAND:
                        stage_band(g0 + 2)
                    if occ == 3 and g0 + 3 < BLOC * NBAND:
                        stage_band(g0 + 3)
                    group(b, bnd0 + 1, occ)
                del vts[g0], vts[g0 + 1], nbs[g0], nbs[g0 + 1]

    nc.compile()
    return nc


_NC_CACHE = None


def _get_nc():
    global _NC_CACHE
    if _NC_CACHE is None:
        _NC_CACHE = build_nc()
    return _NC_CACHE


def kernel(**inputs):
    from concourse.bass_utils import run_bass_kernel_spmd

    nc = _get_nc()
    shard_names = ("x", "w", "noise")
    in_maps = []
    for i in range(NCORES):
        m = {}
        for k, v in inputs.items():
            v = np.ascontiguousarray(np.asarray(v), dtype=np.float32)
            if k in shard_names:
                m[k] = np.ascontiguousarray(v[i * BLOC:(i + 1) * BLOC])
            else:
                m[k] = v
        in_maps.append(m)
    res = run_bass_kernel_spmd(nc, in_maps, list(range(NCORES)))
    outs = [res.results[i]["out"] for i in range(NCORES)]
    return np.concatenate(outs, axis=0).astype(np.float32)


# revision 33
# speedup vs baseline: 1.0968x; 1.0968x over previous
"""Trainium2 Bass kernel for nn_DripBlock: per-sample modulated 3x3 conv.

Math (per sample b):
  s = w @ (linear_w / sqrt(WDIM)).T + linear_b                  [b, in_c]
  base_w = conv_w / sqrt(in_c*3*3)
  wmod = base_w * s[:,None,:,None,None]
  sigma_inv = rsqrt(sum(wmod^2, (in,ky,kx)) + 1e-8)             [b, out]
  y = conv2d(x, wmod*sigma_inv, SAME) + scale_noise*noise + bias
  out = leaky_relu(y, 0.2)

Kernel strategy (data-parallel over batch, 2 samples/core on 8 cores):
  - Fold s into x (xs = x*s per channel); conv against raw conv_w; fold
    C1*sigma_inv, bias, noise into the post ops.
  - 1D Winograd F(2,3) along W; weights stored HALVED (cast scale 0.5)
    so U1=(w0+w1+w2)/2 and U2=(w0-w1+w2)/2 are plain adds of the halved
    taps; the 2x compensation for the U0/U3 planes is folded into the
    drain's scalar_tensor_tensor scalars.
  - Drains use at most one PSUM operand per instruction (PSUM has one
    DVE read port): ScalarE copies M1 to SBUF; DVE computes
    t=2*M0+m1, e=t+M2, t2=m1-M2, o=-2*M3+t2 and z2=ssc*e/o+pre;
    ScalarE computes pre=sn*noise+bias and the leaky-relu (Lrelu
    activation, alpha=0.2) writing interleaved even/odd columns.
  - sigma2[b,oc] = sum_{ic,tap} wchunk^2 s^2 via 36 tiny PE matmuls per
    oc chunk (sqw = ScalarE Square of the resident bf16 weights); PE has
    the slack in sweep0 -- DVE/GpSimd do not.
  - Schedule: occ-OUTER over band pairs so weight emission for occ k+1
    overlaps the two groups of occ k; conv_w rides the scalar HW queue,
    x / xbar-transposes / outputs ride the sync queue; small constant
    vectors are loaded as [4,128] rows and PE-transposed (tiny-gather
    DMAs are slow).  The occ0 prologue interleaves per-icc
    cast/transpose/x-load so the first conv matmul starts ~16us in and
    the per-icc weight chain stays just ahead of the PE.
"""
import numpy as np
from math import sqrt
from contextlib import ExitStack

import concourse.bass as bass
import concourse.bacc as bacc
import concourse.mybir as mybir
import concourse.tile as tile
from concourse.masks import make_identity

B, CIN, COUT, H, W, WDIM, KK = 16, 512, 512, 64, 64, 512, 3
NCORES = 8
BLOC = B // NCORES          # 2 samples per core
P = 128
NIC = CIN // P              # 4 ic chunks
NOC = COUT // P             # 4 oc chunks
NDC = WDIM // P             # 4 wdim chunks
NBAND = 4                   # 16-row bands per sample
RB = H // NBAND             # 16 rows per band
WT = W // 2                 # 32 column tiles (2 output cols each)
XR = RB + 2                 # 18 staged rows per band
EPS = 1e-8
C0 = 1.0 / sqrt(WDIM)
C1 = 1.0 / sqrt(CIN * KK * KK)
SLOPE = 0.2

F32 = mybir.dt.float32
BF16 = mybir.dt.bfloat16
MUL = mybir.AluOpType.mult
ADD = mybir.AluOpType.add
SUB = mybir.AluOpType.subtract
MAX = mybir.AluOpType.max
COPYF = mybir.ActivationFunctionType.Copy
SQRTF = mybir.ActivationFunctionType.Sqrt
IDENTF = mybir.ActivationFunctionType.Identity
LRELUF = mybir.ActivationFunctionType.Lrelu
# The ScalarE Lrelu activation ignores alpha on HW (acts as plain Relu),
# so leaky-relu runs on DVE via max(x, 0.2x).
USE_ACT_LRELU = False


def build_nc():
    nc = bacc.Bacc()

    x_d = nc.declare_dram_parameter("x", [BLOC, CIN, H, W], F32, isOutput=False)
    w_d = nc.declare_dram_parameter("w", [BLOC, WDIM], F32, isOutput=False)
    noise_d = nc.declare_dram_parameter("noise", [BLOC, 1, H, W], F32, isOutput=False)
    lw_d = nc.declare_dram_parameter("linear_w", [CIN, WDIM], F32, isOutput=False)
    lb_d = nc.declare_dram_parameter("linear_b", [CIN], F32, isOutput=False)
    cw_d = nc.declare_dram_parameter("conv_w", [COUT, CIN, KK, KK], F32, isOutput=False)
    sn_d = nc.declare_dram_parameter("scale_noise", [COUT], F32, isOutput=False)
    bias_d = nc.declare_dram_parameter("bias", [COUT], F32, isOutput=False)
    out_d = nc.declare_dram_parameter("out", [BLOC, COUT, H, W], F32, isOutput=True)

    with ExitStack() as ctx:
        tc = ctx.enter_context(tile.TileContext(nc))
        consts = ctx.enter_context(tc.tile_pool(name="consts", bufs=1))
        lw_pool = ctx.enter_context(tc.tile_pool(name="lw", bufs=2))
        lwt_pool = ctx.enter_context(tc.tile_pool(name="lwt", bufs=4))
        co_pool = ctx.enter_context(tc.tile_pool(name="co", bufs=4))
        cobf_pool = ctx.enter_context(tc.tile_pool(name="cobf", bufs=2))
        wt_pool = ctx.enter_context(tc.tile_pool(name="wt", bufs=1))
        u_pool = ctx.enter_context(tc.tile_pool(name="u", bufs=1))
        ua_pool = ctx.enter_context(tc.tile_pool(name="ua", bufs=1))
        sqw_pool = ctx.enter_context(tc.tile_pool(name="sqw", bufs=2))
        small = ctx.enter_context(tc.tile_pool(name="small", bufs=1))
        xt_pool = ctx.enter_context(tc.tile_pool(name="xt", bufs=2))
        v_pool = ctx.enter_context(tc.tile_pool(name="v", bufs=1))
        nb_pool = ctx.enter_context(tc.tile_pool(name="nb", bufs=1))
        dr_pool = ctx.enter_context(tc.tile_pool(name="dr", bufs=2))
        pz_pool = ctx.enter_context(tc.tile_pool(name="pz", bufs=2))
        out_pool = ctx.enter_context(tc.tile_pool(name="out", bufs=2))

        psum = ctx.enter_context(tc.tile_pool(name="mmps", bufs=8, space="PSUM"))

        # ---- conv_w occ0 loads first on the scalar HW queue ----
        co_tiles = {}

        def load_co(occ):
            for icc in range(NIC):
                co = co_pool.tile([P, P * KK * KK], F32, tag="co",
                                  name=f"co{icc}_{occ}")
                nc.scalar.dma_start(
                    out=co,
                    in_=cw_d[occ * P:(occ + 1) * P, icc * P:(icc + 1) * P, :, :]
                    .rearrange("o i a b -> o (i a b)"))
                co_tiles[(icc, occ)] = co

        load_co(0)

        # ---- constants: [4,128] row loads on the sync queue + PE transpose
        # (element-gather DMAs are packet-bound and slow) ----
        ident = consts.tile([P, P], F32)
        make_identity(nc, ident)
        eps_col = consts.tile([P, 1], F32)
        nc.vector.memset(eps_col, EPS)

        # PE warmup: ~4us of dummy matmuls flips the HAM clock gate to 8/8
        # before the real conv stream starts (transposes don't count as
        # PE-busy for HAM).
        wm = consts.tile([P, P], BF16)
        nc.vector.memset(wm, 0.0)
        wm_ps = psum.tile([P, P], F32, tag="mm", name="warm")
        for _ in range(40):
            nc.tensor.matmul(wm_ps, wm, wm, start=True, stop=True)

        # lw on the sync queue so s is ready early
        lw_sbs = []
        for icc in range(NIC):
            lw_sb = lw_pool.tile([P, WDIM], F32, tag="lw", bufs=2,
                                 name=f"lw{icc}")
            nc.sync.dma_start(out=lw_sb, in_=lw_d[icc * P:(icc + 1) * P, :])
            lw_sbs.append(lw_sb)

        rows_pool = ctx.enter_context(tc.tile_pool(name="rows", bufs=1))

        def rowload_cols(src_1d, n, out_ap=None, tag="cols"):
            # src [n*P] -> rows [n, P] -> PE transpose -> [P, n]
            rows = rows_pool.tile([n, P], F32, name="rows", tag="rows")
            nc.sync.dma_start(out=rows, in_=src_1d.rearrange("(c p) -> c p", p=P))
            tp = psum.tile([P, n], F32, tag="mm", name="tpc")
            nc.tensor.transpose(tp, rows, ident[0:n, 0:n])
            if out_ap is None:
                out_ap = consts.tile([P, n], F32, name=tag, tag=tag)
            nc.vector.tensor_copy(out=out_ap, in_=tp)
            return out_ap

        lb_cols = rowload_cols(lb_d[:], NIC, tag="lbc")
        bias_cols = rowload_cols(bias_d[:], NOC, tag="bic")
        sn_cols = rowload_cols(sn_d[:], NOC, tag="snc")
        wcols = consts.tile([P, NDC, BLOC], F32)
        for b in range(BLOC):
            rowload_cols(w_d[b], NDC, out_ap=wcols[:, :, b])

        # ---- phase A: s = w @ (linear_w*C0).T + linear_b, as sT[ic, b] ----
        sT = []
        s2T = []
        for icc in range(NIC):
            lwt = []
            for dc in range(NDC):
                tp = psum.tile([P, P], F32, tag="mm", name="tp")
                nc.tensor.transpose(tp, lw_sbs[icc][:, dc * P:(dc + 1) * P], ident)
                t_ = lwt_pool.tile([P, P], F32, tag="lwt")
                nc.vector.tensor_copy(out=t_, in_=tp)
                lwt.append(t_)
            sp = psum.tile([P, BLOC], F32, tag="mm", name=f"sp{icc}")
            for dc in range(NDC):
                nc.tensor.matmul(sp, lwt[dc], wcols[:, dc, :],
                                 start=(dc == 0), stop=(dc == NDC - 1))
            st = small.tile([P, BLOC], F32, tag=f"sT{icc}")
            nc.vector.tensor_scalar(out=st, in0=sp, scalar1=C0,
                                    scalar2=lb_cols[:, icc:icc + 1],
                                    op0=MUL, op1=ADD)
            s2 = small.tile([P, BLOC], BF16, tag=f"s2T{icc}")
            nc.vector.tensor_mul(s2, st, st)
            sT.append(st)
            s2T.append(s2)

        # ---- staging: x band -> scaled f32 xt -> V planes (bf16) ----
        vts = {}     # gb -> [vt per icc]
        nbs = {}     # gb -> [128, RB*W] f32 noise broadcast

        def stage_icc(gb, icc):
            b, band = divmod(gb, NBAND)
            slot = gb % 3
            r0 = band * RB
            xt = xt_pool.tile([P, XR, W], F32, tag="xt", name=f"xt{gb}_{icc}")
            lo = max(r0 - 1, 0)
            hi = min(r0 - 1 + XR, H)
            j0 = lo - (r0 - 1)
            if j0 > 0:
                nc.vector.memset(xt[:, 0:j0, :], 0.0)
            if (r0 - 1 + XR) > H:
                nc.vector.memset(xt[:, XR - 1:XR, :], 0.0)
            nc.sync.dma_start(
                out=xt[:, j0:j0 + (hi - lo), :],
                in_=x_d[b, icc * P:(icc + 1) * P, lo:hi, :])
            nc.scalar.activation(out=xt, in_=xt, func=COPYF,
                                 scale=sT[icc][:, b:b + 1])
            vt = v_pool.tile([P, 4, XR, WT], BF16, tag=f"v{icc}s{slot}",
                             name=f"v{gb}_{icc}")
            xte = xt.rearrange("p r (w two) -> p r w two", two=2)
            ev = xte[:, :, :, 0]   # x cols 0,2,..,62
            od = xte[:, :, :, 1]   # x cols 1,3,..,63
            # V0/V1 + edge columns on DVE; V2/V3 main bodies on GpSimd
            nc.vector.tensor_tensor(out=vt[:, 0, :, 1:WT], in0=od[:, :, 0:WT - 1],
                                    in1=od[:, :, 1:WT], op=SUB)
            nc.vector.tensor_scalar_mul(out=vt[:, 0, :, 0:1], in0=od[:, :, 0:1],
                                        scalar1=-1.0)
            nc.vector.tensor_tensor(out=vt[:, 1], in0=ev, in1=od, op=ADD)
            nc.gpsimd.tensor_tensor(out=vt[:, 2], in0=od, in1=ev, op=SUB)
            nc.gpsimd.tensor_tensor(out=vt[:, 3, :, 0:WT - 1], in0=ev[:, :, 0:WT - 1],
                                    in1=ev[:, :, 1:WT], op=SUB)
            nc.vector.tensor_copy(out=vt[:, 3, :, WT - 1:WT], in_=ev[:, :, WT - 1:WT])
            return vt

        def stage_band(gb, skip_icc=0):
            b, band = divmod(gb, NBAND)
            vt_l = vts.get(gb, [])
            for icc in range(skip_icc, NIC):
                vt_l.append(stage_icc(gb, icc))
            vts[gb] = vt_l
            r0 = band * RB
            nb = nb_pool.tile([P, RB * W], F32, tag=f"nb{gb % 3}", name=f"nb{gb}")
            nsrc = noise_d[b].rearrange("o h w -> o (h w)")[0:1, r0 * W:(r0 + RB) * W]
            nsrc_bc = bass.AP(tensor=nsrc.tensor, offset=nsrc.offset,
                              ap=[[0, P]] + list(nsrc.ap)[1:])
            nc.scalar.dma_start(out=nb, in_=nsrc_bc)
            nbs[gb] = nb

        # ---- weights: wchunk[ic, tap, oc] (halved), u1/u2, sqw ----
        wchunks = {}
        u1s = {}
        u2s = {}
        sqws = {}
        sig_scale = {}   # occ -> [P, BLOC] f32: C1*sigma_inv

        def emit_weight_chunk(icc, occ):
            co = co_tiles.pop((icc, occ))
            # cast to bf16 in tap-major order with the Winograd 1/2 folded in
            co_bf = cobf_pool.tile([P, KK * KK * P], BF16, tag="cobf")
            nc.scalar.activation(
                out=co_bf.rearrange("o (n i) -> o n i", i=P),
                in_=co.rearrange("o (i n) -> o n i", n=KK * KK),
                func=COPYF, scale=0.5)
            wchunk = wt_pool.tile([P, KK * KK, P], BF16,
                                  tag=f"wt{icc}_{occ}", name=f"wt{icc}_{occ}")
            nc.sync.dma_start_transpose(out=wchunk, in_=co_bf)
            wchunks[(icc, occ)] = wchunk
            # U1/U2 = (g0 +- g1 + g2)/2 of the true taps (halved taps add)
            wc4 = wchunk.rearrange("i (ky kx) o -> i ky kx o", kx=KK)
            w0 = wc4[:, :, 0, :]
            w1 = wc4[:, :, 1, :]
            w2 = wc4[:, :, 2, :]
            ua = ua_pool.tile([P, KK, P], BF16, tag="ua")
            nc.gpsimd.tensor_tensor(out=ua, in0=w0, in1=w2, op=ADD)
            u1 = u_pool.tile([P, KK, P], BF16, tag=f"u1_{icc}_{occ}",
                             name=f"u1_{icc}_{occ}")
            nc.gpsimd.tensor_tensor(out=u1, in0=ua, in1=w1, op=ADD)
            u2 = u_pool.tile([P, KK, P], BF16, tag=f"u2_{icc}_{occ}",
                             name=f"u2_{icc}_{occ}")
            nc.vector.tensor_tensor(out=u2, in0=ua, in1=w1, op=SUB)
            u1s[(icc, occ)] = u1
            u2s[(icc, occ)] = u2

        def emit_sigma_prep(icc, occ):
            # sqw = wchunk^2 (0.25*w^2) on ScalarE (contiguous write)
            sqw = sqw_pool.tile([P, KK * KK, P], BF16, tag="sqw", bufs=4,
                                name=f"sqw{icc}_{occ}")
            nc.scalar.activation(out=sqw, in_=wchunks[(icc, occ)],
                                 func=mybir.ActivationFunctionType.Square)
            sqws[(icc, occ)] = sqw

        def emit_weights(occ):
            for icc in range(NIC):
                emit_weight_chunk(icc, occ)
            if occ + 1 < NOC:
                load_co(occ + 1)

        def emit_sigma(occ):
            # 36 tiny accumulating matmuls: sg[oc,b] = sum_{ic,tap} sqw*s2
            sg = psum.tile([P, BLOC], F32, tag="mm", name=f"sg{occ}")
            for icc in range(NIC):
                sq = sqws.pop((icc, occ))
                for tp_ in range(KK * KK):
                    nc.tensor.matmul(sg, sq[:, tp_, :], s2T[icc],
                                     start=(icc == 0 and tp_ == 0),
                                     stop=(icc == NIC - 1 and tp_ == KK * KK - 1))
            # sg = 0.25 * sum w^2 s^2; sig = sqrt(C1^2 * 4 * sg + EPS)
            sig = small.tile([P, BLOC], F32, tag=f"sig{occ}")
            nc.scalar.activation(out=sig, in_=sg, func=SQRTF,
                                 bias=eps_col[:, 0:1], scale=4.0 * C1 * C1)
            sinv = small.tile([P, BLOC], F32, tag=f"sinv{occ}")
            nc.vector.reciprocal(out=sinv, in_=sig)
            ssc = small.tile([P, BLOC], F32, tag=f"ssc{occ}")
            nc.vector.tensor_scalar_mul(out=ssc, in0=sinv, scalar1=C1)
            sig_scale[occ] = ssc

        # ---- conv group + post ----
        out3 = out_d.rearrange("b c h w -> b c (h w)")

        def group(b, band, occ):
            gb = b * NBAND + band
            vt_l = vts[gb]
            nb_t = nbs[gb]
            M = [psum.tile([P, RB * WT], F32, tag="mm", name=f"M{xi}")
                 for xi in range(4)]
            for icc in range(NIC):
                vt = vt_l[icc]
                wc = wchunks[(icc, occ)]
                for ky in range(KK):
                    lhs = (wc[:, 3 * ky, :],
                           u1s[(icc, occ)][:, ky, :],
                           u2s[(icc, occ)][:, ky, :],
                           wc[:, 3 * ky + 2, :])
                    st = (icc == 0 and ky == 0)
                    sp = (icc == NIC - 1 and ky == KK - 1)
                    for xi in range(4):
                        nc.tensor.matmul(
                            M[xi], lhs[xi], vt[:, xi, ky:ky + RB, :],
                            start=st, stop=sp)
            # drains: one PSUM operand per op.  ScalarE: m1 = M1 -> SBUF.
            m1 = dr_pool.tile([P, RB * WT], F32, tag="m1", name="m1")
            nc.scalar.activation(out=m1, in_=M[1], func=COPYF)
            # DVE: t = 2*M0 + m1 ; e = t + M2 ; t2 = m1 - M2 ; o = -2*M3 + t2
            t = dr_pool.tile([P, RB * WT], F32, tag="t", name="t", bufs=1)
            nc.vector.scalar_tensor_tensor(out=t, in0=M[0], scalar=2.0,
                                           in1=m1, op0=MUL, op1=ADD)
            e = dr_pool.tile([P, RB * WT], F32, tag="e", name="e")
            nc.vector.tensor_tensor(out=e, in0=t, in1=M[2], op=ADD)
            t2 = dr_pool.tile([P, RB * WT], F32, tag="t2", name="t2", bufs=1)
            nc.vector.tensor_tensor(out=t2, in0=m1, in1=M[2], op=SUB)
            o = dr_pool.tile([P, RB * WT], F32, tag="o", name="o")
            nc.vector.scalar_tensor_tensor(out=o, in0=M[3], scalar=-2.0,
                                           in1=t2, op0=MUL, op1=ADD)
            # ScalarE: pre = sn*noise + bias ; DVE: z2 = ssc*e/o + pre
            ssc = sig_scale[occ][:, b:b + 1]
            pre = pz_pool.tile([P, RB, W], BF16, tag="pre", name="pre")
            nc.scalar.activation(out=pre,
                                 in_=nb_t.rearrange("p (r w) -> p r w", r=RB),
                                 func=IDENTF,
                                 scale=sn_cols[:, occ:occ + 1],
                                 bias=bias_cols[:, occ:occ + 1])
            prev = pre.rearrange("p r (w two) -> p r w two", two=2)
            ev3 = e.rearrange("p (r w) -> p r w", w=WT)
            ov3 = o.rearrange("p (r w) -> p r w", w=WT)
            z2e = pz_pool.tile([P, RB, WT], BF16, tag="z2e", name="z2e")
            nc.vector.scalar_tensor_tensor(out=z2e, in0=ev3, scalar=ssc,
                                           in1=prev[:, :, :, 0], op0=MUL, op1=ADD)
            z2o = pz_pool.tile([P, RB, WT], BF16, tag="z2o", name="z2o")
            nc.vector.scalar_tensor_tensor(out=z2o, in0=ov3, scalar=ssc,
                                           in1=prev[:, :, :, 1], op0=MUL, op1=ADD)
            # leaky relu, interleaving even/odd back to row-major
            zo = out_pool.tile([P, RB, WT, 2], F32, tag="zo", name="zo")
            if USE_ACT_LRELU:
                nc.scalar.activation(out=zo[:, :, :, 0], in_=z2e, func=LRELUF,
                                     alpha=SLOPE)
                nc.scalar.activation(out=zo[:, :, :, 1], in_=z2o, func=LRELUF,
                                     alpha=SLOPE)
            else:
                nc.vector.scalar_tensor_tensor(out=zo[:, :, :, 0], in0=z2e,
                                               scalar=SLOPE, in1=z2e,
                                               op0=MUL, op1=MAX)
                nc.vector.scalar_tensor_tensor(out=zo[:, :, :, 1], in0=z2o,
                                               scalar=SLOPE, in1=z2o,
                                               op0=MUL, op1=MAX)
            nc.sync.dma_start(
                out=out3[b, occ * P:(occ + 1) * P,
                         band * RB * W:(band + 1) * RB * W],
                in_=zo.rearrange("p r w two -> p (r w two)"))

        # ---- prologue: interleave occ0 weight chain with band0 staging so
        # the per-icc matmul chain starts as soon as chunk icc lands ----
        vts[0] = []
        for icc in range(NIC):
            vts[0].append(stage_icc(0, icc))
            emit_weight_chunk(icc, 0)
        r0nb = nb_pool.tile([P, RB * W], F32, tag="nb0", name="nb0")
        nsrc = noise_d[0].rearrange("o h w -> o (h w)")[0:1, 0:RB * W]
        nc.scalar.dma_start(out=r0nb, in_=bass.AP(
            tensor=nsrc.tensor, offset=nsrc.offset,
            ap=[[0, P]] + list(nsrc.ap)[1:]))
        nbs[0] = r0nb
        stage_band(1)
        load_co(1)
        for icc in range(NIC):
            emit_sigma_prep(icc, 0)

        def emit_sigma_preps(occ):
            for icc in range(NIC):
                emit_sigma_prep(icc, occ)

        # ---- main loop: occ-outer over band pairs ----
        for b in range(BLOC):
            for pp in range(NBAND // 2):
                bnd0 = 2 * pp
                g0 = b * NBAND + bnd0
                for occ in range(NOC):
                    if b == 0 and pp == 0:
                        emit_sigma(occ)
                    group(b, bnd0, occ)
                    if b == 0 and pp == 0 and occ + 1 < NOC:
                        emit_weights(occ + 1)
                    if occ == 1 and g0 + 2 < BLOC * NBAND:
                        stage_band(g0 + 2)
                    if occ == 3 and g0 + 3 < BLOC * NBAND:
                        stage_band(g0 + 3)
                    group(b, bnd0 + 1, occ)
                    if b == 0 and pp == 0 and occ + 1 < NOC:
                        emit_sigma_preps(occ + 1)
                del vts[g0], vts[g0 + 1], nbs[g0], nbs[g0 + 1]

    nc.compile()
    return nc


_NC_CACHE = None


def _get_nc():
    global _NC_CACHE
    if _NC_CACHE is None:
        _NC_CACHE = build_nc()
    return _NC_CACHE


def kernel(**inputs):
    from concourse.bass_utils import run_bass_kernel_spmd

    nc = _get_nc()
    shard_names = ("x", "w", "noise")
    in_maps = []
    for i in range(NCORES):
        m = {}
        for k, v in inputs.items():
            v = np.ascontiguousarray(np.asarray(v), dtype=np.float32)
            if k in shard_names:
                m[k] = np.ascontiguousarray(v[i * BLOC:(i + 1) * BLOC])
            else:
                m[k] = v
        in_maps.append(m)
    res = run_bass_kernel_spmd(nc, in_maps, list(range(NCORES)))
    outs = [res.results[i]["out"] for i in range(NCORES)]
    return np.concatenate(outs, axis=0).astype(np.float32)
